# revision 21
# baseline (speedup 1.0000x reference)
"""Trainium2 Bass kernel for AlignedContrastiveLoss.

Pipeline (per sample, one NeuronCore each, 8 cores data-parallel over batch):
  1. intensity^2[h,w] = sum_c fmap[c,h,w]^2 via PE Gram blocks (diag extract)
  2. 3x3 local-max peak mask (intensity^2 == maxpool3x3) & (intensity^2 > 0.25)
  3. exact top-10 peaks by intensity (value + linear index)
  4. gather 256-dim features at peak pixels via dynamic-offset DMA
  5. 10x10 cosine similarity, hinge relu(sim - 0.5), masked mean
Host: mean of the 8 per-sample losses.

Works in squared-intensity domain (sqrt is monotonic; threshold 0.5 -> 0.25).
"""

import sys

for _p in ("/opt/trn_rl_repo",):
    if _p not in sys.path:
        sys.path.insert(0, _p)

import numpy as np

import concourse.bass as bass
import concourse.bacc as bacc
import concourse.tile as tile
from concourse import mybir
from concourse.masks import make_identity

F32 = mybir.dt.float32
I32 = mybir.dt.int32
U8 = mybir.dt.uint8
ALU = mybir.AluOpType
ACTF = mybir.ActivationFunctionType
AXX = mybir.AxisListType.X

B = 8
C = 256
H = W = 128
HW = H * W
P = 128
TOPK = 10
CHUNK = 2048           # pixels per DMA chunk
NCHUNK = HW // CHUNK   # 8
NBLK = CHUNK // P      # 16 gram blocks per chunk
BIGNEG = -3.0e38
THRESH2 = 0.25         # 0.5^2
MARGIN = 0.5
EPS = 1e-8


def build_graph(debug_outputs=False, niters=1):
    nc = bacc.Bacc(None)
    det = nc.declare_dram_parameter("det", [C, HW], F32, isOutput=False)
    loc = nc.declare_dram_parameter("loc", [C, HW], F32, isOutput=False)
    out = nc.declare_dram_parameter("out", [1, niters], F32, isOutput=True)
    dbg = {}
    if debug_outputs:
        for nm, shp in [
            ("dbg_i2_det", [P, P]), ("dbg_masked_det", [P, P]),
            ("dbg_vals_det", [1, TOPK]), ("dbg_idx_det", [1, TOPK]),
            ("dbg_vals_loc", [1, TOPK]), ("dbg_idx_loc", [1, TOPK]),
            ("dbg_feats_det", [P, 2 * TOPK]), ("dbg_feats_loc", [P, 2 * TOPK]),
            ("dbg_sim", [TOPK, TOPK]), ("dbg_tot", [1, 2]),
        ]:
            dbg[nm] = nc.declare_dram_parameter(nm, shp, F32, isOutput=True)

    with tile.TileContext(nc) as tc:
        with (
            tc.tile_pool(name="const", bufs=1) as const,
            tc.tile_pool(name="data", bufs=3) as data,
            tc.tile_pool(name="persist", bufs=1) as persist,
            tc.tile_pool(name="work", bufs=2) as work,
            tc.tile_pool(name="gram", bufs=4, space="PSUM") as gramp,
            tc.tile_pool(name="psmall", bufs=2, space="PSUM") as psmall,
        ):
            ident = const.tile([P, P], F32, tag="ident")
            make_identity(nc, ident[:])
            # iota_rev[h, w] = HW - (h*W + w)
            iota_rev_i = const.tile([P, P], I32, tag="iotarevi")
            nc.gpsimd.iota(iota_rev_i[:], pattern=[[-1, P]], base=HW,
                           channel_multiplier=-P)
            iota_rev = const.tile([P, P], F32, tag="iotarev")
            nc.vector.tensor_copy(iota_rev[:], iota_rev_i[:])
            negtile = const.tile([P, P], F32, tag="negtile")
            nc.gpsimd.memset(negtile[:], BIGNEG)
            ones_row = const.tile([1, P], F32, tag="ones_row")
            nc.gpsimd.memset(ones_row[:], 1.0)
            neg_margin = const.tile([P, 1], F32, tag="neg_margin")
            nc.gpsimd.memset(neg_margin[:], -MARGIN)
            zerotile = const.tile([P, P], F32, tag="zerotile")
            nc.gpsimd.memset(zerotile[:], 0.0)

            def compute_masked(src, name):
                """intensity^2 -> peak-masked tile [h, w] (BIGNEG off-peaks)."""
                I2 = persist.tile([P, P], F32, tag=f"I2_{name}")  # [w, h]
                for ch in range(NCHUNK):
                    t0 = data.tile([P, CHUNK], F32, tag="h0")
                    t1 = data.tile([P, CHUNK], F32, tag="h1")
                    sl = slice(ch * CHUNK, (ch + 1) * CHUNK)
                    nc.sync.dma_start(out=t0[:], in_=src[0:P, sl])
                    nc.sync.dma_start(out=t1[:], in_=src[P:C, sl])
                    for b in range(NBLK):
                        col = ch * NBLK + b
                        g = gramp.tile([P, P], F32, tag="gram", space="PSUM")
                        blk = slice(b * P, (b + 1) * P)
                        nc.tensor.matmul(g[:], lhsT=t0[:, blk], rhs=t0[:, blk],
                                         start=True, stop=False)
                        nc.tensor.matmul(g[:], lhsT=t1[:, blk], rhs=t1[:, blk],
                                         start=False, stop=True)
                        scr = work.tile([P, P], F32, tag="scr")
                        nc.vector.tensor_tensor(out=scr[:], in0=g[:],
                                                in1=ident[:], op=ALU.mult)
                        nc.vector.tensor_reduce(out=I2[:, col:col + 1],
                                                in_=scr[:], axis=AXX,
                                                op=ALU.add)

                # 3x3 max pool. I2 is [w, h]: pool along h (free), transpose,
                # pool along w (now free), compare in [h, w] domain.
                E = work.tile([P, P + 2], F32, tag="E")
                nc.gpsimd.memset(E[:], BIGNEG)
                nc.vector.tensor_copy(E[:, 1:P + 1], I2[:])
                cm = work.tile([P, P], F32, tag="cm")
                nc.vector.tensor_tensor(out=cm[:], in0=E[:, 0:P],
                                        in1=E[:, 1:P + 1], op=ALU.max)
                nc.vector.tensor_tensor(out=cm[:], in0=cm[:],
                                        in1=E[:, 2:P + 2], op=ALU.max)
                cmT = psmall.tile([P, P], F32, tag="ps", space="PSUM")
                nc.tensor.transpose(cmT[:], cm[:], ident[:])
                E2 = work.tile([P, P + 2], F32, tag="E2")
                nc.gpsimd.memset(E2[:], BIGNEG)
                nc.vector.tensor_copy(E2[:, 1:P + 1], cmT[:])
                I2T_p = psmall.tile([P, P], F32, tag="ps", space="PSUM")
                nc.tensor.transpose(I2T_p[:], I2[:], ident[:])
                I2T = work.tile([P, P], F32, tag="I2T")
                nc.vector.tensor_copy(I2T[:], I2T_p[:])
                pooled = work.tile([P, P], F32, tag="pooled")
                nc.vector.tensor_tensor(out=pooled[:], in0=E2[:, 0:P],
                                        in1=E2[:, 1:P + 1], op=ALU.max)
                nc.vector.tensor_tensor(out=pooled[:], in0=pooled[:],
                                        in1=E2[:, 2:P + 2], op=ALU.max)
                eqm = work.tile([P, P], U8, tag="eqm")
                nc.vector.tensor_tensor(out=eqm[:], in0=I2T[:], in1=pooled[:],
                                        op=ALU.is_equal)
                gtm = work.tile([P, P], U8, tag="gtm")
                nc.vector.tensor_scalar(out=gtm[:], in0=I2T[:], scalar1=THRESH2,
                                        scalar2=None, op0=ALU.is_gt)
                nc.vector.tensor_tensor(out=eqm[:], in0=eqm[:], in1=gtm[:],
                                        op=ALU.mult)
                masked = persist.tile([P, P], F32, tag=f"masked_{name}")
                nc.vector.tensor_copy(masked[:], negtile[:])
                nc.vector.copy_predicated(masked[:], eqm[:], I2T[:])
                return I2, masked

            def topk(masked, name):
                vals = persist.tile([1, TOPK], F32, tag=f"vals_{name}")
                idxf = persist.tile([1, TOPK], F32, tag=f"idx_{name}")
                for k in range(TOPK):
                    rmax = work.tile([P, 1], F32, tag="rmax")
                    nc.vector.tensor_reduce(out=rmax[:], in_=masked[:],
                                            axis=AXX, op=ALU.max)
                    rmaxT = psmall.tile([1, P], F32, tag="ps", space="PSUM")
                    nc.tensor.transpose(rmaxT[:], rmax[:], ident[:])
                    nc.vector.tensor_reduce(out=vals[:, k:k + 1], in_=rmaxT[:],
                                            axis=AXX, op=ALU.max)
                    # broadcast gmax to [P, 1] via K=1 matmul with ones
                    gbc_p = psmall.tile([P, 1], F32, tag="ps", space="PSUM")
                    nc.tensor.matmul(gbc_p[:], lhsT=ones_row[:],
                                     rhs=vals[:, k:k + 1], start=True,
                                     stop=True)
                    gbc = work.tile([P, 1], F32, tag="gbc")
                    nc.vector.tensor_copy(gbc[:], gbc_p[:])
                    eq2 = work.tile([P, P], U8, tag="eq2")
                    nc.vector.tensor_scalar(out=eq2[:], in0=masked[:],
                                            scalar1=gbc[:, 0:1], scalar2=None,
                                            op0=ALU.is_equal)
                    tsel = work.tile([P, P], F32, tag="tsel")
                    nc.vector.tensor_copy(tsel[:], zerotile[:])
                    nc.vector.copy_predicated(tsel[:], eq2[:], iota_rev[:])
                    rmx = work.tile([P, 1], F32, tag="rmx")
                    nc.vector.tensor_reduce(out=rmx[:], in_=tsel[:],
                                            axis=AXX, op=ALU.max)
                    rmxT = psmall.tile([1, P], F32, tag="ps", space="PSUM")
                    nc.tensor.transpose(rmxT[:], rmx[:], ident[:])
                    grev = work.tile([1, 1], F32, tag="grev")
                    nc.vector.tensor_reduce(out=grev[:], in_=rmxT[:],
                                            axis=AXX, op=ALU.max)
                    # idx = HW - grev = (grev - HW) * -1
                    nc.vector.tensor_scalar(out=idxf[:, k:k + 1], in0=grev[:],
                                            scalar1=float(HW), scalar2=-1.0,
                                            op0=ALU.subtract, op1=ALU.mult)
                    # remove all pixels holding this value
                    nc.vector.copy_predicated(masked[:], eq2[:], negtile[:])
                return vals, idxf

            def gather(src, idxf, name):
                feats = persist.tile([P, 2 * TOPK], F32, tag=f"feats_{name}")
                idxi = persist.tile([1, TOPK], I32, tag=f"idxi_{name}")
                nc.vector.tensor_copy(idxi[:], idxf[:])
                src_v = src[:].rearrange("(j p) hw -> p j hw", p=P)
                feats_v = feats[:].rearrange("p (j k) -> p j k", j=2)
                for k in range(TOPK):
                    reg = nc.alloc_register(mybir.EngineType.Pool,
                                            f"gidx_{name}_{k}_{nc.next_id()}")
                    nc.reg_load(reg, idxi[0:1, k:k + 1])
                    off = nc.snap(reg, donate=True, min_val=0, max_val=HW - 1)
                    nc.gpsimd.dma_start(
                        out=feats_v[:, :, k:k + 1],
                        in_=src_v[:, :, bass.ds(off, 1)])
                return feats

            def cosine_loss(feats_d, feats_l, vals_d, vals_l, out_col):
                T = TOPK
                dots_p = psmall.tile([T, T], F32, tag="cs", space="PSUM")
                nc.tensor.matmul(dots_p[:], lhsT=feats_d[:, 0:T],
                                 rhs=feats_l[:, 0:T], start=True, stop=False)
                nc.tensor.matmul(dots_p[:], lhsT=feats_d[:, T:2 * T],
                                 rhs=feats_l[:, T:2 * T], start=False,
                                 stop=True)
                norms = {}
                for nm, f in (("det", feats_d), ("loc", feats_l)):
                    gg = psmall.tile([T, T], F32, tag="cs", space="PSUM")
                    nc.tensor.matmul(gg[:], lhsT=f[:, 0:T], rhs=f[:, 0:T],
                                     start=True, stop=False)
                    nc.tensor.matmul(gg[:], lhsT=f[:, T:2 * T],
                                     rhs=f[:, T:2 * T], start=False, stop=True)
                    scr = work.tile([T, T], F32, tag="scr1010")
                    n2 = work.tile([T, 1], F32, tag=f"n2_{nm}")
                    nc.vector.tensor_tensor(out=scr[:], in0=gg[:],
                                            in1=ident[0:T, 0:T], op=ALU.mult)
                    nc.vector.tensor_reduce(out=n2[:], in_=scr[:], axis=AXX,
                                            op=ALU.add)
                    na = work.tile([T, 1], F32, tag=f"na_{nm}")
                    nc.scalar.activation(na[:], n2[:], ACTF.Sqrt)
                    nc.vector.tensor_scalar_max(na[:], na[:], EPS)
                    naT_p = psmall.tile([1, T], F32, tag="ps", space="PSUM")
                    nc.tensor.transpose(naT_p[:], na[:], ident[0:T, 0:T])
                    naT = work.tile([1, T], F32, tag=f"naT_{nm}")
                    nc.vector.tensor_copy(naT[:], naT_p[:])
                    norms[nm] = naT
                denom_p = psmall.tile([T, T], F32, tag="cs", space="PSUM")
                nc.tensor.matmul(denom_p[:], lhsT=norms["det"][:],
                                 rhs=norms["loc"][:], start=True, stop=True)
                recip = work.tile([T, T], F32, tag="recip")
                nc.vector.reciprocal(recip[:], denom_p[:])
                sim = work.tile([T, T], F32, tag="sim")
                nc.vector.tensor_tensor(out=sim[:], in0=dots_p[:],
                                        in1=recip[:], op=ALU.mult)
                hinge = work.tile([T, T], F32, tag="hinge")
                nc.scalar.activation(hinge[:], sim[:], ACTF.Relu,
                                     bias=neg_margin[0:T, 0:1])
                vd = work.tile([1, T], F32, tag="vd")
                nc.vector.tensor_scalar(out=vd[:], in0=vals_d[:],
                                        scalar1=THRESH2, scalar2=None,
                                        op0=ALU.is_gt)
                vl = work.tile([1, T], F32, tag="vl")
                nc.vector.tensor_scalar(out=vl[:], in0=vals_l[:],
                                        scalar1=THRESH2, scalar2=None,
                                        op0=ALU.is_gt)
                pmask_p = psmall.tile([T, T], F32, tag="cs", space="PSUM")
                nc.tensor.matmul(pmask_p[:], lhsT=vd[:], rhs=vl[:],
                                 start=True, stop=True)
                pmask = work.tile([T, T], F32, tag="pmask")
                nc.vector.tensor_copy(pmask[:], pmask_p[:])
                rs2 = work.tile([T, 2], F32, tag="rs2")
                scrmh = work.tile([T, T], F32, tag="scrmh")
                nc.vector.tensor_tensor(out=scrmh[:], in0=hinge[:],
                                        in1=pmask[:], op=ALU.mult)
                nc.vector.tensor_reduce(out=rs2[:, 0:1], in_=scrmh[:],
                                        axis=AXX, op=ALU.add)
                nc.vector.tensor_reduce(out=rs2[:, 1:2], in_=pmask[:],
                                        axis=AXX, op=ALU.add)
                rs2T_p = psmall.tile([2, T], F32, tag="ps", space="PSUM")
                nc.tensor.transpose(rs2T_p[:], rs2[:], ident[0:T, 0:T])
                tot2 = work.tile([2, 1], F32, tag="tot2")
                nc.vector.tensor_reduce(out=tot2[:], in_=rs2T_p[:], axis=AXX,
                                        op=ALU.add)
                totT_p = psmall.tile([1, 2], F32, tag="ps", space="PSUM")
                nc.tensor.transpose(totT_p[:], tot2[:], ident[0:2, 0:2])
                tot = work.tile([1, 2], F32, tag="tot")
                nc.vector.tensor_copy(tot[:], totT_p[:])
                # loss = sum / max(n,1) * (n > 0)
                np1 = work.tile([1, 1], F32, tag="np1")
                nc.vector.tensor_scalar_max(np1[:], tot[:, 1:2], 1.0)
                rp = work.tile([1, 1], F32, tag="rp")
                nc.vector.reciprocal(rp[:], np1[:])
                ind = work.tile([1, 1], F32, tag="ind")
                nc.vector.tensor_scalar(out=ind[:], in0=tot[:, 1:2],
                                        scalar1=0.0, scalar2=None,
                                        op0=ALU.is_gt)
                l1 = work.tile([1, 1], F32, tag="l1")
                nc.vector.tensor_tensor(out=l1[:], in0=tot[:, 0:1], in1=rp[:],
                                        op=ALU.mult)
                l2 = work.tile([1, 1], F32, tag="l2")
                nc.vector.tensor_tensor(out=l2[:], in0=l1[:], in1=ind[:],
                                        op=ALU.mult)
                nc.sync.dma_start(out=out[:, out_col:out_col + 1], in_=l2[:])
                return sim, tot

            for it in range(niters):
                I2d, masked_d = compute_masked(det, "det")
                vals_d, idx_d = topk(masked_d, "det")
                feats_d = gather(det, idx_d, "det")
                I2l, masked_l = compute_masked(loc, "loc")
                vals_l, idx_l = topk(masked_l, "loc")
                feats_l = gather(loc, idx_l, "loc")
                sim, tot = cosine_loss(feats_d, feats_l, vals_d, vals_l, it)

            if debug_outputs:
                nc.sync.dma_start(out=dbg["dbg_i2_det"][:], in_=I2d[:])
                nc.sync.dma_start(out=dbg["dbg_masked_det"][:], in_=masked_d[:])
                nc.sync.dma_start(out=dbg["dbg_vals_det"][:], in_=vals_d[:])
                nc.sync.dma_start(out=dbg["dbg_idx_det"][:], in_=idx_d[:])
                nc.sync.dma_start(out=dbg["dbg_vals_loc"][:], in_=vals_l[:])
                nc.sync.dma_start(out=dbg["dbg_idx_loc"][:], in_=idx_l[:])
                nc.sync.dma_start(out=dbg["dbg_feats_det"][:], in_=feats_d[:])
                nc.sync.dma_start(out=dbg["dbg_feats_loc"][:], in_=feats_l[:])
                nc.sync.dma_start(out=dbg["dbg_sim"][:], in_=sim[:])
                nc.sync.dma_start(out=dbg["dbg_tot"][:], in_=tot[:])

    nc.finalize()
    return nc


_CACHED_NC = None


def _get_nc():
    global _CACHED_NC
    if _CACHED_NC is None:
        _CACHED_NC = build_graph(debug_outputs=False, niters=1)
    return _CACHED_NC


def run_spmd(det_b, loc_b, nc=None, **kwargs):
    """det_b/loc_b: [B, C, HW] float32. Returns BassKernelResults."""
    from concourse.bass_utils import run_bass_kernel_spmd

    if nc is None:
        nc = _get_nc()
    in_maps = [
        {"det": np.ascontiguousarray(det_b[b]),
         "loc": np.ascontiguousarray(loc_b[b])}
        for b in range(B)
    ]
    return run_bass_kernel_spmd(nc, in_maps, core_ids=list(range(B)),
                                **kwargs)


def kernel(loc_features, det_features):
    loc_b = np.asarray(loc_features, dtype=np.float32).reshape(B, C, HW)
    det_b = np.asarray(det_features, dtype=np.float32).reshape(B, C, HW)
    res = run_spmd(det_b, loc_b)
    losses = [float(res.results[i]["out"][0, 0]) for i in range(B)]
    return np.asarray(np.sum(losses, dtype=np.float64) / B, dtype=np.float32)


if __name__ == "__main__":
    nc = build_graph()
    print("graph built ok")


# revision 33
# speedup vs baseline: 1552.1754x; 1552.1754x over previous
"""Trainium2 Bass kernel for AlignedContrastiveLoss.

Pipeline (per sample, one NeuronCore each, 8 cores data-parallel over batch):
  1. intensity^2[h,w] = sum_c fmap[c,h,w]^2: ACT squares the streamed data
     (fp32r), PE reduces over channels with an all-ones fp32r matmul
     (1 cyc/row), DVE extracts image rows 8-at-a-time via packed identities
  2. 3x3 local-max peak mask (intensity^2 == maxpool3x3) & (intensity^2 > 0.25)
  3. exact top-10 peaks by intensity (value + linear index)
  4. gather 256-dim features at peak pixels via dynamic-offset DMA
  5. 10x10 cosine similarity, hinge relu(sim - 0.5), masked mean
Host: mean of the 8 per-sample losses.

Works in squared-intensity domain (sqrt is monotonic; threshold 0.5 -> 0.25).
Streaming DMAs alternate between the sync (HWDGE) and gpsimd (SWDGE) queues,
which measures ~10% faster than a single queue. Steady-state per-sample time
measured via an in-NEFF iteration loop: ~100 us vs the 93.8 us HBM roofline
(33.6 MB/core at 358 GB/s).
"""

import sys

for _p in ("/opt/trn_rl_repo",):
    if _p not in sys.path:
        sys.path.insert(0, _p)

import numpy as np

import concourse.bass as bass
import concourse.bacc as bacc
import concourse.tile as tile
from concourse import mybir
from concourse.masks import make_identity

F32 = mybir.dt.float32
F32R = mybir.dt.float32r
I32 = mybir.dt.int32
U8 = mybir.dt.uint8
ALU = mybir.AluOpType
ACTF = mybir.ActivationFunctionType
AXX = mybir.AxisListType.X

B = 8
C = 256
H = W = 128
HW = H * W
P = 128
TOPK = 10
CHUNK = 2048           # pixels per DMA chunk
NCHUNK = HW // CHUNK   # 8
NBLK = CHUNK // P      # 16 gram blocks per chunk
BIGNEG = -3.0e38
THRESH2 = 0.25         # 0.5^2
MARGIN = 0.5
EPS = 1e-8


def build_graph(debug_outputs=False, niters=1):
    nc = bacc.Bacc(None)
    det = nc.declare_dram_parameter("det", [C, HW], F32, isOutput=False)
    loc = nc.declare_dram_parameter("loc", [C, HW], F32, isOutput=False)
    out = nc.declare_dram_parameter("out", [1, niters], F32, isOutput=True)
    dbg = {}
    if debug_outputs:
        for nm, shp in [
            ("dbg_i2_det", [P, P]), ("dbg_masked_det", [P, P]),
            ("dbg_vals_det", [1, TOPK]), ("dbg_idx_det", [1, TOPK]),
            ("dbg_vals_loc", [1, TOPK]), ("dbg_idx_loc", [1, TOPK]),
            ("dbg_feats_det", [P, 2 * TOPK]), ("dbg_feats_loc", [P, 2 * TOPK]),
            ("dbg_sim", [TOPK, TOPK]), ("dbg_tot", [1, 2]),
        ]:
            dbg[nm] = nc.declare_dram_parameter(nm, shp, F32, isOutput=True)

    with tile.TileContext(nc) as tc:
        with (
            tc.tile_pool(name="const", bufs=1) as const,
            tc.tile_pool(name="data", bufs=3) as data,
            tc.tile_pool(name="persist", bufs=1) as persist,
            tc.tile_pool(name="work", bufs=2) as work,
            tc.tile_pool(name="gram", bufs=3, space="PSUM") as gramp,
            tc.tile_pool(name="psmall", bufs=2, space="PSUM") as psmall,
        ):
            ident = const.tile([P, P], F32, tag="ident")
            make_identity(nc, ident[:])
            # 8 identities side by side, for packed diag extraction
            ident8 = const.tile([P, 8 * P], F32, tag="ident8")
            nc.gpsimd.memset(ident8[:], 0.0)
            for j in range(8):
                make_identity(nc, ident8[:, j * P:(j + 1) * P], nomemset=True)
            ones_f = const.tile([P, P], F32, tag="ones_f")
            nc.gpsimd.memset(ones_f[:], 1.0)
            ones_r = const.tile([P, P], F32R, tag="ones_r")
            nc.scalar.activation(ones_r[:], ones_f[:], ACTF.Copy)
            # iota_rev[h, w] = HW - (h*W + w)
            iota_rev_i = const.tile([P, P], I32, tag="iotarevi")
            nc.gpsimd.iota(iota_rev_i[:], pattern=[[-1, P]], base=HW,
                           channel_multiplier=-P)
            iota_rev = const.tile([P, P], F32, tag="iotarev")
            nc.vector.tensor_copy(iota_rev[:], iota_rev_i[:])
            negtile = const.tile([P, P], F32, tag="negtile")
            nc.gpsimd.memset(negtile[:], BIGNEG)
            ones_row = const.tile([1, P], F32, tag="ones_row")
            nc.gpsimd.memset(ones_row[:], 1.0)
            neg_margin = const.tile([P, 1], F32, tag="neg_margin")
            nc.gpsimd.memset(neg_margin[:], -MARGIN)
            zerotile = const.tile([P, P], F32, tag="zerotile")
            nc.gpsimd.memset(zerotile[:], 0.0)

            def compute_masked(src, name):
                """intensity^2 -> peak-masked tile [h, w] (BIGNEG off-peaks).

                Channel reduction: ACT squares the data (fp32r), PE reduces
                over channels with an all-ones fp32r matmul (1 cyc/row), DVE
                extracts 8 image rows per pass via packed identities.
                """
                I2 = persist.tile([P, P], F32, tag=f"I2_{name}")  # [w, h]
                for ch in range(NCHUNK):
                    t0 = data.tile([P, CHUNK], F32, tag="h0")
                    t1 = data.tile([P, CHUNK], F32, tag="h1")
                    sl = slice(ch * CHUNK, (ch + 1) * CHUNK)
                    eng = nc.sync if ch % 2 == 0 else nc.gpsimd
                    eng.dma_start(out=t0[:], in_=src[0:P, sl])
                    eng.dma_start(out=t1[:], in_=src[P:C, sl])
                    sq0 = data.tile([P, CHUNK], F32R, tag="sq0")
                    sq1 = data.tile([P, CHUNK], F32R, tag="sq1")
                    nc.scalar.activation(sq0[:], t0[:], ACTF.Square)
                    nc.scalar.activation(sq1[:], t1[:], ACTF.Square)
                    for grp in range(CHUNK // (8 * P)):  # 1024-px groups
                        g = gramp.tile([P, 8 * P], F32, tag="gram",
                                       space="PSUM")
                        for mh in range(2):  # 512-wide matmul halves
                            msl = slice(grp * 8 * P + mh * 4 * P,
                                        grp * 8 * P + (mh + 1) * 4 * P)
                            osl = slice(mh * 4 * P, (mh + 1) * 4 * P)
                            nc.tensor.matmul(g[:, osl], lhsT=ones_r[:],
                                             rhs=sq0[:, msl],
                                             start=True, stop=False)
                            nc.tensor.matmul(g[:, osl], lhsT=ones_r[:],
                                             rhs=sq1[:, msl],
                                             start=False, stop=True)
                        scr = work.tile([P, 8 * P], F32, tag="scr")
                        nc.vector.tensor_tensor(out=scr[:], in0=g[:],
                                                in1=ident8[:], op=ALU.mult)
                        col = ch * NBLK + grp * 8
                        scr_v = scr[:].rearrange("p (j q) -> p j q", j=8)
                        nc.vector.tensor_reduce(out=I2[:, col:col + 8],
                                                in_=scr_v, axis=AXX,
                                                op=ALU.add)

                # 3x3 max pool. I2 is [w, h]: pool along h (free), transpose,
                # pool along w (now free), compare in [h, w] domain.
                E = work.tile([P, P + 2], F32, tag="E")
                nc.gpsimd.memset(E[:], BIGNEG)
                nc.vector.tensor_copy(E[:, 1:P + 1], I2[:])
                cm = work.tile([P, P], F32, tag="cm")
                nc.vector.tensor_tensor(out=cm[:], in0=E[:, 0:P],
                                        in1=E[:, 1:P + 1], op=ALU.max)
                nc.vector.tensor_tensor(out=cm[:], in0=cm[:],
                                        in1=E[:, 2:P + 2], op=ALU.max)
                cmT = psmall.tile([P, P], F32, tag="ps", space="PSUM")
                nc.tensor.transpose(cmT[:], cm[:], ident[:])
                E2 = work.tile([P, P + 2], F32, tag="E2")
                nc.gpsimd.memset(E2[:], BIGNEG)
                nc.vector.tensor_copy(E2[:, 1:P + 1], cmT[:])
                I2T_p = psmall.tile([P, P], F32, tag="ps", space="PSUM")
                nc.tensor.transpose(I2T_p[:], I2[:], ident[:])
                I2T = work.tile([P, P], F32, tag="I2T")
                nc.vector.tensor_copy(I2T[:], I2T_p[:])
                pooled = work.tile([P, P], F32, tag="pooled")
                nc.vector.tensor_tensor(out=pooled[:], in0=E2[:, 0:P],
                                        in1=E2[:, 1:P + 1], op=ALU.max)
                nc.vector.tensor_tensor(out=pooled[:], in0=pooled[:],
                                        in1=E2[:, 2:P + 2], op=ALU.max)
                eqm = work.tile([P, P], U8, tag="eqm")
                nc.vector.tensor_tensor(out=eqm[:], in0=I2T[:], in1=pooled[:],
                                        op=ALU.is_equal)
                gtm = work.tile([P, P], U8, tag="gtm")
                nc.vector.tensor_scalar(out=gtm[:], in0=I2T[:], scalar1=THRESH2,
                                        scalar2=None, op0=ALU.is_gt)
                nc.vector.tensor_tensor(out=eqm[:], in0=eqm[:], in1=gtm[:],
                                        op=ALU.mult)
                masked = persist.tile([P, P], F32, tag=f"masked_{name}")
                nc.vector.tensor_copy(masked[:], negtile[:])
                nc.vector.copy_predicated(masked[:], eqm[:], I2T[:])
                return I2, masked

            def topk(masked, name):
                vals = persist.tile([1, TOPK], F32, tag=f"vals_{name}")
                idxf = persist.tile([1, TOPK], F32, tag=f"idx_{name}")
                for k in range(TOPK):
                    rmax = work.tile([P, 1], F32, tag="rmax")
                    nc.vector.tensor_reduce(out=rmax[:], in_=masked[:],
                                            axis=AXX, op=ALU.max)
                    rmaxT = psmall.tile([1, P], F32, tag="ps", space="PSUM")
                    nc.tensor.transpose(rmaxT[:], rmax[:], ident[:])
                    nc.vector.tensor_reduce(out=vals[:, k:k + 1], in_=rmaxT[:],
                                            axis=AXX, op=ALU.max)
                    # broadcast gmax to [P, 1] via K=1 matmul with ones
                    gbc_p = psmall.tile([P, 1], F32, tag="ps", space="PSUM")
                    nc.tensor.matmul(gbc_p[:], lhsT=ones_row[:],
                                     rhs=vals[:, k:k + 1], start=True,
                                     stop=True)
                    gbc = work.tile([P, 1], F32, tag="gbc")
                    nc.vector.tensor_copy(gbc[:], gbc_p[:])
                    eq2 = work.tile([P, P], U8, tag="eq2")
                    nc.vector.tensor_scalar(out=eq2[:], in0=masked[:],
                                            scalar1=gbc[:, 0:1], scalar2=None,
                                            op0=ALU.is_equal)
                    tsel = work.tile([P, P], F32, tag="tsel")
                    nc.vector.tensor_copy(tsel[:], zerotile[:])
                    nc.vector.copy_predicated(tsel[:], eq2[:], iota_rev[:])
                    rmx = work.tile([P, 1], F32, tag="rmx")
                    nc.vector.tensor_reduce(out=rmx[:], in_=tsel[:],
                                            axis=AXX, op=ALU.max)
                    rmxT = psmall.tile([1, P], F32, tag="ps", space="PSUM")
                    nc.tensor.transpose(rmxT[:], rmx[:], ident[:])
                    grev = work.tile([1, 1], F32, tag="grev")
                    nc.vector.tensor_reduce(out=grev[:], in_=rmxT[:],
                                            axis=AXX, op=ALU.max)
                    # idx = HW - grev = (grev - HW) * -1
                    nc.vector.tensor_scalar(out=idxf[:, k:k + 1], in0=grev[:],
                                            scalar1=float(HW), scalar2=-1.0,
                                            op0=ALU.subtract, op1=ALU.mult)
                    # remove all pixels holding this value
                    nc.vector.copy_predicated(masked[:], eq2[:], negtile[:])
                return vals, idxf

            def gather(src, idxf, name):
                feats = persist.tile([P, 2 * TOPK], F32, tag=f"feats_{name}")
                idxi = persist.tile([1, TOPK], I32, tag=f"idxi_{name}")
                nc.vector.tensor_copy(idxi[:], idxf[:])
                src_v = src[:].rearrange("(j p) hw -> p j hw", p=P)
                feats_v = feats[:].rearrange("p (j k) -> p j k", j=2)
                for k in range(TOPK):
                    reg = nc.alloc_register(mybir.EngineType.Pool,
                                            f"gidx_{name}_{k}_{nc.next_id()}")
                    nc.reg_load(reg, idxi[0:1, k:k + 1])
                    off = nc.snap(reg, donate=True, min_val=0, max_val=HW - 1)
                    nc.gpsimd.dma_start(
                        out=feats_v[:, :, k:k + 1],
                        in_=src_v[:, :, bass.ds(off, 1)])
                return feats

            def cosine_loss(feats_d, feats_l, vals_d, vals_l, out_col):
                T = TOPK
                dots_p = psmall.tile([T, T], F32, tag="ps", space="PSUM")
                nc.tensor.matmul(dots_p[:], lhsT=feats_d[:, 0:T],
                                 rhs=feats_l[:, 0:T], start=True, stop=False)
                nc.tensor.matmul(dots_p[:], lhsT=feats_d[:, T:2 * T],
                                 rhs=feats_l[:, T:2 * T], start=False,
                                 stop=True)
                norms = {}
                for nm, f in (("det", feats_d), ("loc", feats_l)):
                    gg = psmall.tile([T, T], F32, tag="ps", space="PSUM")
                    nc.tensor.matmul(gg[:], lhsT=f[:, 0:T], rhs=f[:, 0:T],
                                     start=True, stop=False)
                    nc.tensor.matmul(gg[:], lhsT=f[:, T:2 * T],
                                     rhs=f[:, T:2 * T], start=False, stop=True)
                    scr = work.tile([T, T], F32, tag="scr1010")
                    n2 = work.tile([T, 1], F32, tag=f"n2_{nm}")
                    nc.vector.tensor_tensor(out=scr[:], in0=gg[:],
                                            in1=ident[0:T, 0:T], op=ALU.mult)
                    nc.vector.tensor_reduce(out=n2[:], in_=scr[:], axis=AXX,
                                            op=ALU.add)
                    na = work.tile([T, 1], F32, tag=f"na_{nm}")
                    nc.scalar.activation(na[:], n2[:], ACTF.Sqrt)
                    nc.vector.tensor_scalar_max(na[:], na[:], EPS)
                    naT_p = psmall.tile([1, T], F32, tag="ps", space="PSUM")
                    nc.tensor.transpose(naT_p[:], na[:], ident[0:T, 0:T])
                    naT = work.tile([1, T], F32, tag=f"naT_{nm}")
                    nc.vector.tensor_copy(naT[:], naT_p[:])
                    norms[nm] = naT
                denom_p = psmall.tile([T, T], F32, tag="ps", space="PSUM")
                nc.tensor.matmul(denom_p[:], lhsT=norms["det"][:],
                                 rhs=norms["loc"][:], start=True, stop=True)
                recip = work.tile([T, T], F32, tag="recip")
                nc.vector.reciprocal(recip[:], denom_p[:])
                sim = work.tile([T, T], F32, tag="sim")
                nc.vector.tensor_tensor(out=sim[:], in0=dots_p[:],
                                        in1=recip[:], op=ALU.mult)
                hinge = work.tile([T, T], F32, tag="hinge")
                nc.scalar.activation(hinge[:], sim[:], ACTF.Relu,
                                     bias=neg_margin[0:T, 0:1])
                vd = work.tile([1, T], F32, tag="vd")
                nc.vector.tensor_scalar(out=vd[:], in0=vals_d[:],
                                        scalar1=THRESH2, scalar2=None,
                                        op0=ALU.is_gt)
                vl = work.tile([1, T], F32, tag="vl")
                nc.vector.tensor_scalar(out=vl[:], in0=vals_l[:],
                                        scalar1=THRESH2, scalar2=None,
                                        op0=ALU.is_gt)
                pmask_p = psmall.tile([T, T], F32, tag="ps", space="PSUM")
                nc.tensor.matmul(pmask_p[:], lhsT=vd[:], rhs=vl[:],
                                 start=True, stop=True)
                pmask = work.tile([T, T], F32, tag="pmask")
                nc.vector.tensor_copy(pmask[:], pmask_p[:])
                rs2 = work.tile([T, 2], F32, tag="rs2")
                scrmh = work.tile([T, T], F32, tag="scrmh")
                nc.vector.tensor_tensor(out=scrmh[:], in0=hinge[:],
                                        in1=pmask[:], op=ALU.mult)
                nc.vector.tensor_reduce(out=rs2[:, 0:1], in_=scrmh[:],
                                        axis=AXX, op=ALU.add)
                nc.vector.tensor_reduce(out=rs2[:, 1:2], in_=pmask[:],
                                        axis=AXX, op=ALU.add)
                rs2T_p = psmall.tile([2, T], F32, tag="ps", space="PSUM")
                nc.tensor.transpose(rs2T_p[:], rs2[:], ident[0:T, 0:T])
                tot2 = work.tile([2, 1], F32, tag="tot2")
                nc.vector.tensor_reduce(out=tot2[:], in_=rs2T_p[:], axis=AXX,
                                        op=ALU.add)
                totT_p = psmall.tile([1, 2], F32, tag="ps", space="PSUM")
                nc.tensor.transpose(totT_p[:], tot2[:], ident[0:2, 0:2])
                tot = work.tile([1, 2], F32, tag="tot")
                nc.vector.tensor_copy(tot[:], totT_p[:])
                # loss = sum / max(n,1) * (n > 0)
                np1 = work.tile([1, 1], F32, tag="np1")
                nc.vector.tensor_scalar_max(np1[:], tot[:, 1:2], 1.0)
                rp = work.tile([1, 1], F32, tag="rp")
                nc.vector.reciprocal(rp[:], np1[:])
                ind = work.tile([1, 1], F32, tag="ind")
                nc.vector.tensor_scalar(out=ind[:], in0=tot[:, 1:2],
                                        scalar1=0.0, scalar2=None,
                                        op0=ALU.is_gt)
                l1 = work.tile([1, 1], F32, tag="l1")
                nc.vector.tensor_tensor(out=l1[:], in0=tot[:, 0:1], in1=rp[:],
                                        op=ALU.mult)
                l2 = work.tile([1, 1], F32, tag="l2")
                nc.vector.tensor_tensor(out=l2[:], in0=l1[:], in1=ind[:],
                                        op=ALU.mult)
                nc.sync.dma_start(out=out[:, out_col:out_col + 1], in_=l2[:])
                return sim, tot

            for it in range(niters):
                I2d, masked_d = compute_masked(det, "det")
                vals_d, idx_d = topk(masked_d, "det")
                feats_d = gather(det, idx_d, "det")
                I2l, masked_l = compute_masked(loc, "loc")
                vals_l, idx_l = topk(masked_l, "loc")
                feats_l = gather(loc, idx_l, "loc")
                sim, tot = cosine_loss(feats_d, feats_l, vals_d, vals_l, it)

            if debug_outputs:
                nc.sync.dma_start(out=dbg["dbg_i2_det"][:], in_=I2d[:])
                nc.sync.dma_start(out=dbg["dbg_masked_det"][:], in_=masked_d[:])
                nc.sync.dma_start(out=dbg["dbg_vals_det"][:], in_=vals_d[:])
                nc.sync.dma_start(out=dbg["dbg_idx_det"][:], in_=idx_d[:])
                nc.sync.dma_start(out=dbg["dbg_vals_loc"][:], in_=vals_l[:])
                nc.sync.dma_start(out=dbg["dbg_idx_loc"][:], in_=idx_l[:])
                nc.sync.dma_start(out=dbg["dbg_feats_det"][:], in_=feats_d[:])
                nc.sync.dma_start(out=dbg["dbg_feats_loc"][:], in_=feats_l[:])
                nc.sync.dma_start(out=dbg["dbg_sim"][:], in_=sim[:])
                nc.sync.dma_start(out=dbg["dbg_tot"][:], in_=tot[:])

    nc.finalize()
    return nc


_CACHED_NC = None


def _get_nc():
    global _CACHED_NC
    if _CACHED_NC is None:
        _CACHED_NC = build_graph(debug_outputs=False, niters=1)
    return _CACHED_NC


def run_spmd(det_b, loc_b, nc=None, **kwargs):
    """det_b/loc_b: [B, C, HW] float32. Returns BassKernelResults."""
    from concourse.bass_utils import run_bass_kernel_spmd

    if nc is None:
        nc = _get_nc()
    in_maps = [
        {"det": np.ascontiguousarray(det_b[b]),
         "loc": np.ascontiguousarray(loc_b[b])}
        for b in range(B)
    ]
    return run_bass_kernel_spmd(nc, in_maps, core_ids=list(range(B)),
                                **kwargs)


def kernel(loc_features, det_features):
    loc_b = np.asarray(loc_features, dtype=np.float32).reshape(B, C, HW)
    det_b = np.asarray(det_features, dtype=np.float32).reshape(B, C, HW)
    res = run_spmd(det_b, loc_b)
    losses = [float(res.results[i]["out"][0, 0]) for i in range(B)]
    return np.asarray(np.sum(losses, dtype=np.float64) / B, dtype=np.float32)


if __name__ == "__main__":
    nc = build_graph()
    print("graph built ok")


# revision 38
# speedup vs baseline: 1772.3878x; 1.1419x over previous
"""Trainium2 Bass kernel for AlignedContrastiveLoss.

Pipeline (per sample, one NeuronCore each, 8 cores data-parallel over batch):
  1. intensity^2[h,w] = sum_c fmap[c,h,w]^2: ACT squares the streamed data
     (fp32r), PE reduces over channels with an all-ones fp32r matmul
     (1 cyc/row), DVE extracts image rows 8-at-a-time via packed identities
  2. 3x3 local-max peak mask (intensity^2 == maxpool3x3) & (intensity^2 > 0.25)
  3. exact top-10 peaks by intensity (value + linear index)
  4. gather 256-dim features at peak pixels via dynamic-offset DMA
  5. 10x10 cosine similarity, hinge relu(sim - 0.5), masked mean
Host: mean of the 8 per-sample losses.

Works in squared-intensity domain (sqrt is monotonic; threshold 0.5 -> 0.25).
Streaming DMAs alternate between the sync (HWDGE) and gpsimd (SWDGE) queues,
which measures ~10% faster than a single queue. Steady-state per-sample time
measured via an in-NEFF iteration loop: ~100 us vs the 93.8 us HBM roofline
(33.6 MB/core at 358 GB/s).
"""

import sys

for _p in ("/opt/trn_rl_repo",):
    if _p not in sys.path:
        sys.path.insert(0, _p)

import numpy as np

import concourse.bass as bass
import concourse.bacc as bacc
import concourse.tile as tile
from concourse import mybir
from concourse.masks import make_identity

F32 = mybir.dt.float32
F32R = mybir.dt.float32r
I32 = mybir.dt.int32
U8 = mybir.dt.uint8
ALU = mybir.AluOpType
ACTF = mybir.ActivationFunctionType
AXX = mybir.AxisListType.X

B = 8
C = 256
H = W = 128
HW = H * W
P = 128
TOPK = 10
CHUNK = 2048           # pixels per DMA chunk
NCHUNK = HW // CHUNK   # 8
NBLK = CHUNK // P      # 16 gram blocks per chunk
BIGNEG = -3.0e38
THRESH2 = 0.25         # 0.5^2
MARGIN = 0.5
EPS = 1e-8


def build_graph(debug_outputs=False, niters=1):
    nc = bacc.Bacc(None)
    det = nc.declare_dram_parameter("det", [C, HW], F32, isOutput=False)
    loc = nc.declare_dram_parameter("loc", [C, HW], F32, isOutput=False)
    out = nc.declare_dram_parameter("out", [1, niters], F32, isOutput=True)
    dbg = {}
    if debug_outputs:
        for nm, shp in [
            ("dbg_i2_det", [P, P]), ("dbg_masked_det", [P, P]),
            ("dbg_vals_det", [1, TOPK]), ("dbg_idx_det", [1, TOPK]),
            ("dbg_vals_loc", [1, TOPK]), ("dbg_idx_loc", [1, TOPK]),
            ("dbg_feats_det", [P, 2 * TOPK]), ("dbg_feats_loc", [P, 2 * TOPK]),
            ("dbg_sim", [TOPK, TOPK]), ("dbg_tot", [1, 2]),
        ]:
            dbg[nm] = nc.declare_dram_parameter(nm, shp, F32, isOutput=True)

    with tile.TileContext(nc) as tc:
        with (
            tc.tile_pool(name="const", bufs=1) as const,
            tc.tile_pool(name="data", bufs=3) as data,
            tc.tile_pool(name="persist", bufs=1) as persist,
            tc.tile_pool(name="work", bufs=2) as work,
            tc.tile_pool(name="gram", bufs=3, space="PSUM") as gramp,
            tc.tile_pool(name="psmall", bufs=2, space="PSUM") as psmall,
        ):
            ident = const.tile([P, P], F32, tag="ident")
            make_identity(nc, ident[:])
            # 8 identities side by side, for packed diag extraction
            ident8 = const.tile([P, 8 * P], F32, tag="ident8")
            nc.gpsimd.memset(ident8[:], 0.0)
            for j in range(8):
                make_identity(nc, ident8[:, j * P:(j + 1) * P], nomemset=True)
            ones_f = const.tile([P, P], F32, tag="ones_f")
            nc.gpsimd.memset(ones_f[:], 1.0)
            ones_r = const.tile([P, P], F32R, tag="ones_r")
            nc.scalar.activation(ones_r[:], ones_f[:], ACTF.Copy)
            # iota_rev[h, w] = HW - (h*W + w)
            iota_rev_i = const.tile([P, P], I32, tag="iotarevi")
            nc.gpsimd.iota(iota_rev_i[:], pattern=[[-1, P]], base=HW,
                           channel_multiplier=-P)
            iota_rev = const.tile([P, P], F32, tag="iotarev")
            nc.vector.tensor_copy(iota_rev[:], iota_rev_i[:])
            negtile = const.tile([P, P], F32, tag="negtile")
            nc.gpsimd.memset(negtile[:], BIGNEG)
            ones_row = const.tile([1, P], F32, tag="ones_row")
            nc.gpsimd.memset(ones_row[:], 1.0)
            neg_margin = const.tile([P, 1], F32, tag="neg_margin")
            nc.gpsimd.memset(neg_margin[:], -MARGIN)
            zerotile = const.tile([P, P], F32, tag="zerotile")
            nc.gpsimd.memset(zerotile[:], 0.0)

            def compute_masked(src, name):
                """intensity^2 -> peak-masked tile [h, w] (BIGNEG off-peaks).

                Channel reduction: ACT squares the data (fp32r), PE reduces
                over channels with an all-ones fp32r matmul (1 cyc/row), DVE
                extracts 8 image rows per pass via packed identities.
                """
                I2 = persist.tile([P, P], F32, tag=f"I2_{name}")  # [w, h]
                for ch in range(NCHUNK):
                    t0 = data.tile([P, CHUNK], F32, tag="h0")
                    t1 = data.tile([P, CHUNK], F32, tag="h1")
                    sl = slice(ch * CHUNK, (ch + 1) * CHUNK)
                    eng = (nc.sync, nc.gpsimd)[ch % 2]
                    eng.dma_start(out=t0[:], in_=src[0:P, sl])
                    eng.dma_start(out=t1[:], in_=src[P:C, sl])
                    sq0 = data.tile([P, CHUNK], F32R, tag="sq0")
                    sq1 = data.tile([P, CHUNK], F32R, tag="sq1")
                    nc.scalar.activation(sq0[:], t0[:], ACTF.Square)
                    nc.scalar.activation(sq1[:], t1[:], ACTF.Square)
                    for grp in range(CHUNK // (8 * P)):  # 1024-px groups
                        g = gramp.tile([P, 8 * P], F32, tag="gram",
                                       space="PSUM")
                        for mh in range(2):  # 512-wide matmul halves
                            msl = slice(grp * 8 * P + mh * 4 * P,
                                        grp * 8 * P + (mh + 1) * 4 * P)
                            osl = slice(mh * 4 * P, (mh + 1) * 4 * P)
                            nc.tensor.matmul(g[:, osl], lhsT=ones_r[:],
                                             rhs=sq0[:, msl],
                                             start=True, stop=False)
                            nc.tensor.matmul(g[:, osl], lhsT=ones_r[:],
                                             rhs=sq1[:, msl],
                                             start=False, stop=True)
                        scr = work.tile([P, 8 * P], F32, tag="scr")
                        nc.vector.tensor_tensor(out=scr[:], in0=g[:],
                                                in1=ident8[:], op=ALU.mult)
                        col = ch * NBLK + grp * 8
                        scr_v = scr[:].rearrange("p (j q) -> p j q", j=8)
                        nc.vector.tensor_reduce(out=I2[:, col:col + 8],
                                                in_=scr_v, axis=AXX,
                                                op=ALU.add)

                # 3x3 max pool. I2 is [w, h]: pool along h (free), transpose,
                # pool along w (now free), compare in [h, w] domain.
                E = work.tile([P, P + 2], F32, tag="E")
                nc.gpsimd.memset(E[:], BIGNEG)
                nc.vector.tensor_copy(E[:, 1:P + 1], I2[:])
                cm = work.tile([P, P], F32, tag="cm")
                nc.vector.tensor_tensor(out=cm[:], in0=E[:, 0:P],
                                        in1=E[:, 1:P + 1], op=ALU.max)
                nc.vector.tensor_tensor(out=cm[:], in0=cm[:],
                                        in1=E[:, 2:P + 2], op=ALU.max)
                cmT = psmall.tile([P, P], F32, tag="ps", space="PSUM")
                nc.tensor.transpose(cmT[:], cm[:], ident[:])
                E2 = work.tile([P, P + 2], F32, tag="E2")
                nc.gpsimd.memset(E2[:], BIGNEG)
                nc.vector.tensor_copy(E2[:, 1:P + 1], cmT[:])
                I2T_p = psmall.tile([P, P], F32, tag="ps", space="PSUM")
                nc.tensor.transpose(I2T_p[:], I2[:], ident[:])
                I2T = work.tile([P, P], F32, tag="I2T")
                nc.vector.tensor_copy(I2T[:], I2T_p[:])
                pooled = work.tile([P, P], F32, tag="pooled")
                nc.vector.tensor_tensor(out=pooled[:], in0=E2[:, 0:P],
                                        in1=E2[:, 1:P + 1], op=ALU.max)
                nc.vector.tensor_tensor(out=pooled[:], in0=pooled[:],
                                        in1=E2[:, 2:P + 2], op=ALU.max)
                # NOTE: the >THRESH2 term of the reference's peak mask is
                # omitted here on purpose: top-k ordering is by value, and
                # validity is re-checked as vals > THRESH2 downstream, so the
                # selected valid set (and the loss) is provably identical.
                eqm = work.tile([P, P], U8, tag="eqm")
                nc.vector.tensor_tensor(out=eqm[:], in0=I2T[:], in1=pooled[:],
                                        op=ALU.is_equal)
                masked = persist.tile([P, P], F32, tag=f"masked_{name}")
                nc.vector.tensor_copy(masked[:], negtile[:])
                nc.vector.copy_predicated(masked[:], eqm[:], I2T[:])
                return I2, masked

            def topk(masked, name):
                vals = persist.tile([1, TOPK], F32, tag=f"vals_{name}")
                idxf = persist.tile([1, TOPK], F32, tag=f"idx_{name}")
                for k in range(TOPK):
                    rmax = work.tile([P, 1], F32, tag="rmax")
                    nc.vector.tensor_reduce(out=rmax[:], in_=masked[:],
                                            axis=AXX, op=ALU.max)
                    rmaxT = psmall.tile([1, P], F32, tag="ps", space="PSUM")
                    nc.tensor.transpose(rmaxT[:], rmax[:], ident[:])
                    nc.vector.tensor_reduce(out=vals[:, k:k + 1], in_=rmaxT[:],
                                            axis=AXX, op=ALU.max)
                    # broadcast gmax to [P, 1] via K=1 matmul with ones
                    gbc_p = psmall.tile([P, 1], F32, tag="ps", space="PSUM")
                    nc.tensor.matmul(gbc_p[:], lhsT=ones_row[:],
                                     rhs=vals[:, k:k + 1], start=True,
                                     stop=True)
                    gbc = work.tile([P, 1], F32, tag="gbc")
                    nc.vector.tensor_copy(gbc[:], gbc_p[:])
                    eq2 = work.tile([P, P], U8, tag="eq2")
                    nc.vector.tensor_scalar(out=eq2[:], in0=masked[:],
                                            scalar1=gbc[:, 0:1], scalar2=None,
                                            op0=ALU.is_equal)
                    tsel = work.tile([P, P], F32, tag="tsel")
                    nc.vector.tensor_tensor(out=tsel[:], in0=eq2[:],
                                            in1=iota_rev[:], op=ALU.mult)
                    rmx = work.tile([P, 1], F32, tag="rmx")
                    nc.vector.tensor_reduce(out=rmx[:], in_=tsel[:],
                                            axis=AXX, op=ALU.max)
                    rmxT = psmall.tile([1, P], F32, tag="ps", space="PSUM")
                    nc.tensor.transpose(rmxT[:], rmx[:], ident[:])
                    grev = work.tile([1, 1], F32, tag="grev")
                    nc.vector.tensor_reduce(out=grev[:], in_=rmxT[:],
                                            axis=AXX, op=ALU.max)
                    # idx = HW - grev = (grev - HW) * -1
                    nc.vector.tensor_scalar(out=idxf[:, k:k + 1], in0=grev[:],
                                            scalar1=float(HW), scalar2=-1.0,
                                            op0=ALU.subtract, op1=ALU.mult)
                    # remove all pixels holding this value
                    nc.vector.copy_predicated(masked[:], eq2[:], negtile[:])
                return vals, idxf

            def gather(src, idxf, name):
                feats = persist.tile([P, 2 * TOPK], F32, tag=f"feats_{name}")
                idxi = persist.tile([1, TOPK], I32, tag=f"idxi_{name}")
                nc.vector.tensor_copy(idxi[:], idxf[:])
                src_v = src[:].rearrange("(j p) hw -> p j hw", p=P)
                feats_v = feats[:].rearrange("p (j k) -> p j k", j=2)
                for k in range(TOPK):
                    reg = nc.alloc_register(mybir.EngineType.Pool,
                                            f"gidx_{name}_{k}_{nc.next_id()}")
                    nc.reg_load(reg, idxi[0:1, k:k + 1])
                    off = nc.snap(reg, donate=True, min_val=0, max_val=HW - 1)
                    nc.gpsimd.dma_start(
                        out=feats_v[:, :, k:k + 1],
                        in_=src_v[:, :, bass.ds(off, 1)])
                return feats

            def cosine_loss(feats_d, feats_l, vals_d, vals_l, out_col):
                T = TOPK
                dots_p = psmall.tile([T, T], F32, tag="ps", space="PSUM")
                nc.tensor.matmul(dots_p[:], lhsT=feats_d[:, 0:T],
                                 rhs=feats_l[:, 0:T], start=True, stop=False)
                nc.tensor.matmul(dots_p[:], lhsT=feats_d[:, T:2 * T],
                                 rhs=feats_l[:, T:2 * T], start=False,
                                 stop=True)
                norms = {}
                for nm, f in (("det", feats_d), ("loc", feats_l)):
                    gg = psmall.tile([T, T], F32, tag="ps", space="PSUM")
                    nc.tensor.matmul(gg[:], lhsT=f[:, 0:T], rhs=f[:, 0:T],
                                     start=True, stop=False)
                    nc.tensor.matmul(gg[:], lhsT=f[:, T:2 * T],
                                     rhs=f[:, T:2 * T], start=False, stop=True)
                    scr = work.tile([T, T], F32, tag="scr1010")
                    n2 = work.tile([T, 1], F32, tag=f"n2_{nm}")
                    nc.vector.tensor_tensor(out=scr[:], in0=gg[:],
                                            in1=ident[0:T, 0:T], op=ALU.mult)
                    nc.vector.tensor_reduce(out=n2[:], in_=scr[:], axis=AXX,
                                            op=ALU.add)
                    na = work.tile([T, 1], F32, tag=f"na_{nm}")
                    nc.scalar.activation(na[:], n2[:], ACTF.Sqrt)
                    nc.vector.tensor_scalar_max(na[:], na[:], EPS)
                    naT_p = psmall.tile([1, T], F32, tag="ps", space="PSUM")
                    nc.tensor.transpose(naT_p[:], na[:], ident[0:T, 0:T])
                    naT = work.tile([1, T], F32, tag=f"naT_{nm}")
                    nc.vector.tensor_copy(naT[:], naT_p[:])
                    norms[nm] = naT
                denom_p = psmall.tile([T, T], F32, tag="ps", space="PSUM")
                nc.tensor.matmul(denom_p[:], lhsT=norms["det"][:],
                                 rhs=norms["loc"][:], start=True, stop=True)
                recip = work.tile([T, T], F32, tag="recip")
                nc.vector.reciprocal(recip[:], denom_p[:])
                sim = work.tile([T, T], F32, tag="sim")
                nc.vector.tensor_tensor(out=sim[:], in0=dots_p[:],
                                        in1=recip[:], op=ALU.mult)
                hinge = work.tile([T, T], F32, tag="hinge")
                nc.scalar.activation(hinge[:], sim[:], ACTF.Relu,
                                     bias=neg_margin[0:T, 0:1])
                vd = work.tile([1, T], F32, tag="vd")
                nc.vector.tensor_scalar(out=vd[:], in0=vals_d[:],
                                        scalar1=THRESH2, scalar2=None,
                                        op0=ALU.is_gt)
                vl = work.tile([1, T], F32, tag="vl")
                nc.vector.tensor_scalar(out=vl[:], in0=vals_l[:],
                                        scalar1=THRESH2, scalar2=None,
                                        op0=ALU.is_gt)
                pmask_p = psmall.tile([T, T], F32, tag="ps", space="PSUM")
                nc.tensor.matmul(pmask_p[:], lhsT=vd[:], rhs=vl[:],
                                 start=True, stop=True)
                pmask = work.tile([T, T], F32, tag="pmask")
                nc.vector.tensor_copy(pmask[:], pmask_p[:])
                rs2 = work.tile([T, 2], F32, tag="rs2")
                scrmh = work.tile([T, T], F32, tag="scrmh")
                nc.vector.tensor_tensor(out=scrmh[:], in0=hinge[:],
                                        in1=pmask[:], op=ALU.mult)
                nc.vector.tensor_reduce(out=rs2[:, 0:1], in_=scrmh[:],
                                        axis=AXX, op=ALU.add)
                nc.vector.tensor_reduce(out=rs2[:, 1:2], in_=pmask[:],
                                        axis=AXX, op=ALU.add)
                rs2T_p = psmall.tile([2, T], F32, tag="ps", space="PSUM")
                nc.tensor.transpose(rs2T_p[:], rs2[:], ident[0:T, 0:T])
                tot2 = work.tile([2, 1], F32, tag="tot2")
                nc.vector.tensor_reduce(out=tot2[:], in_=rs2T_p[:], axis=AXX,
                                        op=ALU.add)
                totT_p = psmall.tile([1, 2], F32, tag="ps", space="PSUM")
                nc.tensor.transpose(totT_p[:], tot2[:], ident[0:2, 0:2])
                tot = work.tile([1, 2], F32, tag="tot")
                nc.vector.tensor_copy(tot[:], totT_p[:])
                # loss = sum / max(n,1) * (n > 0)
                np1 = work.tile([1, 1], F32, tag="np1")
                nc.vector.tensor_scalar_max(np1[:], tot[:, 1:2], 1.0)
                rp = work.tile([1, 1], F32, tag="rp")
                nc.vector.reciprocal(rp[:], np1[:])
                ind = work.tile([1, 1], F32, tag="ind")
                nc.vector.tensor_scalar(out=ind[:], in0=tot[:, 1:2],
                                        scalar1=0.0, scalar2=None,
                                        op0=ALU.is_gt)
                l1 = work.tile([1, 1], F32, tag="l1")
                nc.vector.tensor_tensor(out=l1[:], in0=tot[:, 0:1], in1=rp[:],
                                        op=ALU.mult)
                l2 = work.tile([1, 1], F32, tag="l2")
                nc.vector.tensor_tensor(out=l2[:], in0=l1[:], in1=ind[:],
                                        op=ALU.mult)
                nc.sync.dma_start(out=out[:, out_col:out_col + 1], in_=l2[:])
                return sim, tot

            for it in range(niters):
                I2d, masked_d = compute_masked(det, "det")
                vals_d, idx_d = topk(masked_d, "det")
                feats_d = gather(det, idx_d, "det")
                I2l, masked_l = compute_masked(loc, "loc")
                vals_l, idx_l = topk(masked_l, "loc")
                feats_l = gather(loc, idx_l, "loc")
                sim, tot = cosine_loss(feats_d, feats_l, vals_d, vals_l, it)

            if debug_outputs:
                nc.sync.dma_start(out=dbg["dbg_i2_det"][:], in_=I2d[:])
                nc.sync.dma_start(out=dbg["dbg_masked_det"][:], in_=masked_d[:])
                nc.sync.dma_start(out=dbg["dbg_vals_det"][:], in_=vals_d[:])
                nc.sync.dma_start(out=dbg["dbg_idx_det"][:], in_=idx_d[:])
                nc.sync.dma_start(out=dbg["dbg_vals_loc"][:], in_=vals_l[:])
                nc.sync.dma_start(out=dbg["dbg_idx_loc"][:], in_=idx_l[:])
                nc.sync.dma_start(out=dbg["dbg_feats_det"][:], in_=feats_d[:])
                nc.sync.dma_start(out=dbg["dbg_feats_loc"][:], in_=feats_l[:])
                nc.sync.dma_start(out=dbg["dbg_sim"][:], in_=sim[:])
                nc.sync.dma_start(out=dbg["dbg_tot"][:], in_=tot[:])

    nc.finalize()
    return nc


_CACHED_NC = None


def _get_nc():
    global _CACHED_NC
    if _CACHED_NC is None:
        _CACHED_NC = build_graph(debug_outputs=False, niters=1)
    return _CACHED_NC


def run_spmd(det_b, loc_b, nc=None, **kwargs):
    """det_b/loc_b: [B, C, HW] float32. Returns BassKernelResults."""
    from concourse.bass_utils import run_bass_kernel_spmd

    if nc is None:
        nc = _get_nc()
    in_maps = [
        {"det": np.ascontiguousarray(det_b[b]),
         "loc": np.ascontiguousarray(loc_b[b])}
        for b in range(B)
    ]
    return run_bass_kernel_spmd(nc, in_maps, core_ids=list(range(B)),
                                **kwargs)


def kernel(loc_features, det_features):
    loc_b = np.asarray(loc_features, dtype=np.float32).reshape(B, C, HW)
    det_b = np.asarray(det_features, dtype=np.float32).reshape(B, C, HW)
    res = run_spmd(det_b, loc_b)
    losses = [float(res.results[i]["out"][0, 0]) for i in range(B)]
    return np.asarray(np.sum(losses, dtype=np.float64) / B, dtype=np.float32)


if __name__ == "__main__":
    nc = build_graph()
    print("graph built ok")
